# revision 20
# baseline (speedup 1.0000x reference)
"""Trainium2 Bass kernel for an attention layer.

Computes, for each batch element b:
    q      = x @ W                  [T, D]
    scores = q @ x^T                [T, T]
    out    = softmax(scores) @ x    [T, D]

with B=8, T=4096, D=64, f32 in/out. Sharding: data-parallel over batch,
one batch element per NeuronCore (8 cores), W replicated. No collectives.

Per-core algorithm (flash-style, scores never touch HBM):
  - x rows mapped to SBUF partition p = t // 32 (contiguous 8KB DMA
    descriptors per partition; the induced row permutation is applied
    identically to the s axis, the t axis and the output, so it cancels).
  - xT [128, T] bf16 via PE transposes; partitions 64-127 duplicate
    0-63 (SBUF->SBUF DMA) so score matmuls run as 2 concurrent K=64
    matmuls (PE row tiling).
  - qT [128, T] bf16 = W^T-stationary matmul over xT, same duplication.
  - x_aug [128, blk, 65] bf16 with a ones column (row sums come free).
  - per 512-col panel of t, per pair of 128-row s-blocks:
      scoresT pair -> PSUM f32 [128, 1024]
      exp -> SBUF bf16: ScalarE (table exp) for most pairs, VectorE for
      the rest via a fused Schraudolph: i16 = round(s*128*log2e +
      128*(127-C)) is exactly the bit pattern of bf16 2^(s*log2e - C),
      one tensor_scalar instruction per pair.
      o_augT[65, 512] += x_aug_blk^T @ expT_blk (PSUM f32, accumulated)
    accum matmuls are emitted one pair behind the score matmuls so the
    in-order PE queue never head-blocks on the exp engines.
    epilogue: PE-transpose to [128, 65], divide by the sums column,
    stage into an SBUF accumulator; one contiguous output DMA at the end.
"""

import numpy as np

B, T, D = 8, 4096, 64
P = 128                 # SBUF/PSUM partitions
NBLK = T // P           # 32 row blocks of s
PW = 512                # panel width (t columns per panel)
NPANEL = T // PW        # 8 panels
DA = D + 1              # augmented with ones column
NPAIR = NBLK // 2       # 16 block pairs per panel

# Schraudolph constants: i16 = round(s * SCHRAUD_A + SCHRAUD_B) viewed as
# bf16 is 2^(s*log2e - C) ~= exp(s). C trades max error for mean error.
LOG2E = 1.4426950408889634
SCHRAUD_C = 0.0570
SCHRAUD_A = 128.0 * LOG2E
SCHRAUD_B = 128.0 * (127.0 - SCHRAUD_C)

# Which of the 16 pairs per panel the DVE handles (rest go to ScalarE);
# MIXED_PAIRS are split: ScalarE takes the first block, DVE the second.
DVE_PAIRS = frozenset({1, 4, 7, 9, 11, 13})
MIXED_PAIRS = frozenset({14})


def build_bass(stage="full", dve_pairs=DVE_PAIRS, mixed_pairs=MIXED_PAIRS,
               repeat=1):
    import concourse.bacc as bacc
    import concourse.mybir as mybir
    import concourse.tile as tile
    from concourse.masks import make_identity

    f32 = mybir.dt.float32
    bf16 = mybir.dt.bfloat16
    i16 = mybir.dt.int16
    EXP = mybir.ActivationFunctionType.Exp
    MULT = mybir.AluOpType.mult
    ADD = mybir.AluOpType.add

    nc = bacc.Bacc("TRN2", target_bir_lowering=False, debug=False, num_devices=B)

    x_ext = nc.dram_tensor("x", [T, D], f32, kind="ExternalInput")
    w_ext = nc.dram_tensor("W", [D, D], f32, kind="ExternalInput")
    out_ext = nc.dram_tensor("out", [T, D], f32, kind="ExternalOutput")

    # row t = p*NBLK + j: partition p's rows are contiguous in DRAM, so the
    # in/out DMAs are 128 descriptors of 8KB instead of 4096 of 256B.
    x_view = x_ext.ap().rearrange("(p j) d -> p j d", p=P)
    out_view = out_ext.ap().rearrange("(p j) d -> p (j d)", p=P)

    with tile.TileContext(nc) as tc:
        with (
            tc.tile_pool(name="const", bufs=1) as const,
            tc.tile_pool(name="sb", bufs=1) as sb,
            tc.tile_pool(name="sc_ps", bufs=3, space="PSUM") as sc_ps,
            tc.tile_pool(name="o_ps", bufs=1, space="PSUM") as o_ps,
            tc.tile_pool(name="tp2_ps", bufs=1, space="PSUM") as tp2_ps,
            tc.tile_pool(name="exps", bufs=4) as exps,
            tc.tile_pool(name="small", bufs=4) as small,
        ):
            ident = const.tile([P, P], f32)
            make_identity(nc, ident[:])

            x_sb = sb.tile([P, NBLK, D], f32)       # x rows on partitions
            w_sb = const.tile([D, D], f32)
            w_bf = const.tile([D, D], bf16)
            x_aug = sb.tile([P, NBLK, DA], bf16)
            xT = sb.tile([P, T], bf16)
            qT = sb.tile([P, T], bf16)
            osb_all = sb.tile([P, NBLK, D], f32)    # staged output rows

            panels = [] if stage == "prologue" else (
                [0] if stage == "panel1" else list(range(NPANEL)))

            def emit_scores(pnl, g):
                """Score matmuls + exp for pair g of panel pnl."""
                sc = sc_ps.tile([P, 2 * PW], f32, tag="sc")
                for h in range(2):
                    k = 2 * g + h
                    base = D * (k % 2)
                    nc.tensor.matmul(
                        sc[:, h * PW:(h + 1) * PW],
                        xT[base:base + D, k * P:(k + 1) * P],
                        qT[base:base + D, pnl * PW:(pnl + 1) * PW],
                        start=True, stop=True,
                    )
                ex = exps.tile([P, 2 * PW], bf16, tag="ex")
                if g in dve_pairs:
                    nc.vector.tensor_scalar(
                        out=ex[:].bitcast(i16), in0=sc[:],
                        scalar1=float(SCHRAUD_A), scalar2=float(SCHRAUD_B),
                        op0=MULT, op1=ADD,
                    )
                elif g in mixed_pairs:
                    nc.scalar.activation(ex[:, 0:PW], sc[:, 0:PW], EXP)
                    nc.vector.tensor_scalar(
                        out=ex[:, PW:2 * PW].bitcast(i16), in0=sc[:, PW:2 * PW],
                        scalar1=float(SCHRAUD_A), scalar2=float(SCHRAUD_B),
                        op0=MULT, op1=ADD,
                    )
                else:
                    nc.scalar.activation(ex[:], sc[:], EXP)
                return ex

            def emit_accum(g, ex, op):
                for h in range(2):
                    k = 2 * g + h
                    nc.tensor.matmul(
                        op[:],
                        x_aug[:, k, :],
                        ex[:, h * PW:(h + 1) * PW],
                        start=(k == 0), stop=(k == NBLK - 1),
                    )

            def emit_epilogue(pnl, ob):
                """Transpose + normalize + stage panel pnl's output."""
                tp2 = tp2_ps.tile([P, 4, DA], f32, tag="tp2")
                for j2 in range(4):
                    nc.tensor.transpose(
                        tp2[:, j2, :], ob[:, j2 * P:(j2 + 1) * P],
                        ident[0:DA, 0:DA],
                    )
                for j2 in range(4):
                    jj = pnl * 4 + j2
                    rc = small.tile([P, 1], f32, tag="rc")
                    nc.vector.reciprocal(rc[:], tp2[:, j2, D:DA])
                    nc.vector.tensor_scalar(
                        out=osb_all[:, jj, :], in0=tp2[:, j2, 0:D],
                        scalar1=rc[:], scalar2=None, op0=MULT,
                    )

            for rep in range(repeat):
                # x on the SP HWDGE queue (2 chunks so transposes can start
                # early), W on the ACT queue so the loads run in parallel.
                nc.sync.dma_start(out=x_sb[:, 0:NBLK // 2, :],
                                  in_=x_view[:, 0:NBLK // 2, :])
                nc.sync.dma_start(out=x_sb[:, NBLK // 2:NBLK, :],
                                  in_=x_view[:, NBLK // 2:NBLK, :])
                nc.scalar.dma_start(out=w_sb[:], in_=w_ext.ap())
                nc.vector.tensor_copy(w_bf[:], w_sb[:])

                # x_aug: [P, NBLK, DA] bf16 with ones in the last column
                nc.vector.memset(x_aug[:, :, D:DA], 1.0)
                half = NBLK // 2
                nc.vector.tensor_copy(x_aug[:, 0:half, 0:D], x_sb[:, 0:half, :])
                nc.scalar.copy(x_aug[:, half:NBLK, 0:D], x_sb[:, half:NBLK, :])

                # xT [128, T] bf16 via PE transposes (rows 0-63), then
                # SBUF->SBUF DMAs duplicate onto rows 64-127 for row tiling.
                # 8 transposed blocks per PSUM tile -> one [64,1024] copy.
                for r in range(NBLK // 8):
                    tp = sc_ps.tile([P, 2 * PW], f32, tag="sc")
                    for j in range(8):
                        blk = 8 * r + j
                        nc.tensor.transpose(
                            tp[0:D, j * P:(j + 1) * P], x_sb[:, blk, :],
                            ident[:],
                        )
                    sl = slice(r * 8 * P, (r + 1) * 8 * P)
                    if r % 2 == 0:
                        nc.vector.tensor_copy(xT[0:D, sl], tp[0:D, :])
                    else:
                        nc.scalar.copy(xT[0:D, sl], tp[0:D, :])
                    nc.sync.dma_start(out=xT[D:2 * D, sl], in_=xT[0:D, sl])

                # qT [128, T] bf16 = W^T @ xT (stationary W, K = D),
                # two panels per PSUM tile -> one [64,1024] copy.
                for jp in range(NPANEL // 2):
                    qp = sc_ps.tile([P, 2 * PW], f32, tag="sc")
                    for h in range(2):
                        j = 2 * jp + h
                        nc.tensor.matmul(
                            qp[0:D, h * PW:(h + 1) * PW], w_bf[:],
                            xT[0:D, j * PW:(j + 1) * PW],
                            start=True, stop=True,
                        )
                    sl = slice(2 * jp * PW, (2 * jp + 2) * PW)
                    if jp % 2 == 0:
                        nc.vector.tensor_copy(qT[0:D, sl], qp[0:D, :])
                    else:
                        nc.scalar.copy(qT[0:D, sl], qp[0:D, :])
                    nc.sync.dma_start(out=qT[D:2 * D, sl], in_=qT[0:D, sl])

                if stage == "prologue":
                    out_dbg = out_ext.ap().rearrange("(a b) d -> a (b d)", a=D)
                    nc.gpsimd.dma_start(out=out_dbg, in_=qT[0:D, :])

                prev = None  # (pnl, ob) pending epilogue
                for pnl in panels:
                    op = o_ps.tile([DA, PW], f32, tag="o")
                    exq = []
                    for g in range(NPAIR):
                        exq.append(emit_scores(pnl, g))
                        if g == 1 and prev is not None:
                            emit_epilogue(*prev)
                            prev = None
                        if g >= 1:
                            emit_accum(g - 1, exq[g - 1], op)
                    emit_accum(NPAIR - 1, exq[NPAIR - 1], op)
                    ob = small.tile([DA, PW], f32, tag="ob")
                    nc.vector.tensor_copy(ob[:], op[:])
                    prev = (pnl, ob)
                if prev is not None:
                    emit_epilogue(*prev)

                if stage != "prologue":
                    nc.sync.dma_start(
                        out=out_view,
                        in_=osb_all[:].rearrange("p j d -> p (j d)"),
                    )

    if not nc.is_finalized():
        nc.finalize()
    return nc


def kernel(inputs: np.ndarray, W: np.ndarray) -> np.ndarray:
    from concourse.bass_utils import run_bass_kernel_spmd

    nc = build_bass()
    x = np.ascontiguousarray(np.asarray(inputs, dtype=np.float32))
    w = np.ascontiguousarray(np.asarray(W, dtype=np.float32))
    in_maps = [{"x": x[i], "W": w} for i in range(B)]
    res = run_bass_kernel_spmd(nc, in_maps, core_ids=list(range(B)))
    out = np.stack([res.results[i]["out"] for i in range(B)], axis=0)
    return out.astype(np.float32)


if __name__ == "__main__":
    rng = np.random.default_rng(0)
    x = rng.standard_normal((B, T, D), dtype=np.float32)
    w = (rng.standard_normal((D, D)) * 0.05).astype(np.float32)
    out = kernel(inputs=x, W=w)
    print("out", out.shape, out.dtype)


# revision 27
# speedup vs baseline: 1.1316x; 1.1316x over previous
"""Trainium2 Bass kernel for an attention layer.

Computes, for each batch element b:
    q      = x @ W                  [T, D]
    scores = q @ x^T                [T, T]
    out    = softmax(scores) @ x    [T, D]

with B=8, T=4096, D=64, f32 in/out. Sharding: data-parallel over batch,
one batch element per NeuronCore (8 cores), W replicated. No collectives.

Per-core algorithm (flash-style, scores never touch HBM):
  - x rows mapped to SBUF partition p = t // 32 (contiguous 8KB DMA
    descriptors per partition; the induced row permutation is applied
    identically to the s axis, the t axis and the output, so it cancels).
  - xT [128, T] bf16 via PE transposes; partitions 64-127 duplicate
    0-63 (SBUF->SBUF DMA) so score matmuls run as 2 concurrent K=64
    matmuls (PE row tiling).
  - qT [128, T] bf16 = W^T-stationary matmul over xT, same duplication.
  - x_aug [128, blk, 65] bf16 with a ones column (row sums come free).
  - per 512-col panel of t, per pair of 128-row s-blocks:
      scoresT pair -> PSUM f32 [128, 1024]
      exp -> SBUF bf16: ScalarE (table exp) for most pairs, VectorE for
      the rest via a fused Schraudolph: i16 = round(s*128*log2e +
      128*(127-C)) is exactly the bit pattern of bf16 2^(s*log2e - C),
      one tensor_scalar instruction per pair.
      o_augT[65, 512] += x_aug_blk^T @ expT_blk (PSUM f32, accumulated)
    accum matmuls are emitted one pair behind the score matmuls so the
    in-order PE queue never head-blocks on the exp engines.
    epilogue: PE-transpose to [128, 65], divide by the sums column,
    stage into an SBUF accumulator; one contiguous output DMA at the end.
"""

import numpy as np

B, T, D = 8, 4096, 64
P = 128                 # SBUF/PSUM partitions
NBLK = T // P           # 32 row blocks of s
PW = 512                # panel width (t columns per panel)
NPANEL = T // PW        # 8 panels
DA = D + 1              # augmented with ones column
NPAIR = NBLK // 2       # 16 block pairs per panel

# Schraudolph constants: i16 = round(s * SCHRAUD_A + SCHRAUD_B) viewed as
# bf16 is 2^(s*log2e - C) ~= exp(s). C trades max error for mean error.
LOG2E = 1.4426950408889634
SCHRAUD_C = 0.0570
SCHRAUD_A = 128.0 * LOG2E
SCHRAUD_B = 128.0 * (127.0 - SCHRAUD_C)

# Which of the 16 pairs per panel the DVE handles (rest go to ScalarE);
# MIXED_PAIRS are split: ScalarE takes the first block, DVE the second.
DVE_PAIRS = frozenset({1, 4, 7, 9, 11, 13})
MIXED_PAIRS = frozenset({14})


def build_bass(stage="full", dve_pairs=DVE_PAIRS, mixed_pairs=MIXED_PAIRS,
               repeat=1):
    import concourse.bacc as bacc
    import concourse.mybir as mybir
    import concourse.tile as tile
    from concourse.masks import make_identity

    f32 = mybir.dt.float32
    bf16 = mybir.dt.bfloat16
    i16 = mybir.dt.int16
    EXP = mybir.ActivationFunctionType.Exp
    MULT = mybir.AluOpType.mult
    ADD = mybir.AluOpType.add

    nc = bacc.Bacc("TRN2", target_bir_lowering=False, debug=False, num_devices=B)

    x_ext = nc.dram_tensor("x", [T, D], f32, kind="ExternalInput")
    w_ext = nc.dram_tensor("W", [D, D], f32, kind="ExternalInput")
    out_ext = nc.dram_tensor("out", [T, D], f32, kind="ExternalOutput")

    # row t = p*NBLK + j: partition p's rows are contiguous in DRAM, so the
    # in/out DMAs are 128 descriptors of 8KB instead of 4096 of 256B.
    x_view = x_ext.ap().rearrange("(p j) d -> p j d", p=P)
    out_view = out_ext.ap().rearrange("(p j) d -> p (j d)", p=P)

    with tile.TileContext(nc) as tc:
        with (
            tc.tile_pool(name="const", bufs=1) as const,
            tc.tile_pool(name="sb", bufs=1) as sb,
            tc.tile_pool(name="sc_ps", bufs=3, space="PSUM") as sc_ps,
            tc.tile_pool(name="o_ps", bufs=1, space="PSUM") as o_ps,
            tc.tile_pool(name="tp2_ps", bufs=1, space="PSUM") as tp2_ps,
            tc.tile_pool(name="exps", bufs=6) as exps,
            tc.tile_pool(name="small", bufs=4) as small,
        ):
            ident = const.tile([P, P], f32)
            make_identity(nc, ident[:])

            x_sb = sb.tile([P, NBLK, D], f32)       # x rows on partitions
            w_sb = const.tile([D, D], f32)
            w_bf = const.tile([D, D], bf16)
            x_aug = sb.tile([P, NBLK, DA], bf16)
            xT = sb.tile([P, T], bf16)
            qT = sb.tile([P, T], bf16)
            osb_all = sb.tile([P, NBLK, D], f32)    # staged output rows

            panels = [] if stage == "prologue" else (
                [0] if stage == "panel1" else list(range(NPANEL)))

            def emit_scores(pnl, g):
                """Score matmuls + exp for pair g of panel pnl."""
                sc = sc_ps.tile([P, 2 * PW], f32, tag="sc")
                for h in range(2):
                    k = 2 * g + h
                    base = D * (k % 2)
                    nc.tensor.matmul(
                        sc[:, h * PW:(h + 1) * PW],
                        xT[base:base + D, k * P:(k + 1) * P],
                        qT[base:base + D, pnl * PW:(pnl + 1) * PW],
                        start=True, stop=True,
                    )
                ex = exps.tile([P, 2 * PW], bf16, tag="ex")
                if g in dve_pairs:
                    nc.vector.tensor_scalar(
                        out=ex[:].bitcast(i16), in0=sc[:],
                        scalar1=float(SCHRAUD_A), scalar2=float(SCHRAUD_B),
                        op0=MULT, op1=ADD,
                    )
                elif g in mixed_pairs:
                    nc.scalar.activation(ex[:, 0:PW], sc[:, 0:PW], EXP)
                    nc.vector.tensor_scalar(
                        out=ex[:, PW:2 * PW].bitcast(i16), in0=sc[:, PW:2 * PW],
                        scalar1=float(SCHRAUD_A), scalar2=float(SCHRAUD_B),
                        op0=MULT, op1=ADD,
                    )
                else:
                    nc.scalar.activation(ex[:], sc[:], EXP)
                return ex

            def emit_accum(g, ex, op):
                for h in range(2):
                    k = 2 * g + h
                    nc.tensor.matmul(
                        op[:],
                        x_aug[:, k, :],
                        ex[:, h * PW:(h + 1) * PW],
                        start=(k == 0), stop=(k == NBLK - 1),
                    )

            def emit_epilogue(pnl, ob):
                """Transpose + normalize + stage panel pnl's output."""
                tp2 = tp2_ps.tile([P, 4, DA], f32, tag="tp2")
                for j2 in range(4):
                    nc.tensor.transpose(
                        tp2[:, j2, :], ob[:, j2 * P:(j2 + 1) * P],
                        ident[0:DA, 0:DA],
                    )
                for j2 in range(4):
                    jj = pnl * 4 + j2
                    rc = small.tile([P, 1], f32, tag="rc")
                    nc.vector.reciprocal(rc[:], tp2[:, j2, D:DA])
                    nc.vector.tensor_scalar(
                        out=osb_all[:, jj, :], in0=tp2[:, j2, 0:D],
                        scalar1=rc[:], scalar2=None, op0=MULT,
                    )

            for rep in range(repeat):
                # x on the SP HWDGE queue (one DMA: HBM descriptors have a
                # large fixed cost), W on the ACT queue so the loads overlap.
                nc.sync.dma_start(out=x_sb[:], in_=x_view)
                nc.scalar.dma_start(out=w_sb[:], in_=w_ext.ap())
                nc.vector.tensor_copy(w_bf[:], w_sb[:])

                # x_aug: [P, NBLK, DA] bf16 with ones in the last column
                nc.vector.memset(x_aug[:, :, D:DA], 1.0)
                half = NBLK // 2
                nc.vector.tensor_copy(x_aug[:, 0:half, 0:D], x_sb[:, 0:half, :])
                nc.scalar.copy(x_aug[:, half:NBLK, 0:D], x_sb[:, half:NBLK, :])

                # xT [128, T] bf16 via PE transposes (rows 0-63), then
                # SBUF->SBUF DMAs duplicate onto rows 64-127 for row tiling.
                # 8 transposed blocks per PSUM tile -> one [64,1024] copy.
                for r in range(NBLK // 8):
                    tp = sc_ps.tile([P, 2 * PW], f32, tag="sc")
                    for j in range(8):
                        blk = 8 * r + j
                        nc.tensor.transpose(
                            tp[0:D, j * P:(j + 1) * P], x_sb[:, blk, :],
                            ident[:],
                        )
                    sl = slice(r * 8 * P, (r + 1) * 8 * P)
                    if r % 2 == 0:
                        nc.vector.tensor_copy(xT[0:D, sl], tp[0:D, :])
                    else:
                        nc.scalar.copy(xT[0:D, sl], tp[0:D, :])
                    nc.sync.dma_start(out=xT[D:2 * D, sl], in_=xT[0:D, sl])

                # qT [128, T] bf16 = W^T @ xT (stationary W, K = D),
                # two panels per PSUM tile -> one [64,1024] copy.
                for jp in range(NPANEL // 2):
                    qp = sc_ps.tile([P, 2 * PW], f32, tag="sc")
                    for h in range(2):
                        j = 2 * jp + h
                        nc.tensor.matmul(
                            qp[0:D, h * PW:(h + 1) * PW], w_bf[:],
                            xT[0:D, j * PW:(j + 1) * PW],
                            start=True, stop=True,
                        )
                    sl = slice(2 * jp * PW, (2 * jp + 2) * PW)
                    if jp % 2 == 0:
                        nc.vector.tensor_copy(qT[0:D, sl], qp[0:D, :])
                    else:
                        nc.scalar.copy(qT[0:D, sl], qp[0:D, :])
                    nc.sync.dma_start(out=qT[D:2 * D, sl], in_=qT[0:D, sl])

                if stage == "prologue":
                    out_dbg = out_ext.ap().rearrange("(a b) d -> a (b d)", a=D)
                    nc.gpsimd.dma_start(out=out_dbg, in_=qT[0:D, :])

                prev = None  # (pnl, ob) pending epilogue
                for pnl in panels:
                    op = o_ps.tile([DA, PW], f32, tag="o")
                    exq = []
                    for g in range(NPAIR):
                        exq.append(emit_scores(pnl, g))
                        if g == 1 and prev is not None:
                            done = prev[0]
                            emit_epilogue(*prev)
                            prev = None
                            if stage == "full" and done == NPANEL - 3:
                                # panels 0..5 staged: store them on the ACT
                                # ring while the last panels compute (the x
                                # load owns the SP ring)
                                nc.scalar.dma_start(
                                    out=out_view[:, 0:24 * D],
                                    in_=osb_all[:, 0:24, :]
                                    .rearrange("p j d -> p (j d)"),
                                )
                        if g >= 1:
                            emit_accum(g - 1, exq[g - 1], op)
                    emit_accum(NPAIR - 1, exq[NPAIR - 1], op)
                    ob = small.tile([DA, PW], f32, tag="ob")
                    nc.vector.tensor_copy(ob[:], op[:])
                    prev = (pnl, ob)
                if prev is not None:
                    emit_epilogue(*prev)

                if stage == "full":
                    nc.scalar.dma_start(
                        out=out_view[:, 24 * D:32 * D],
                        in_=osb_all[:, 24:32, :].rearrange("p j d -> p (j d)"),
                    )
                elif stage != "prologue":
                    nc.scalar.dma_start(
                        out=out_view,
                        in_=osb_all[:].rearrange("p j d -> p (j d)"),
                    )

    if not nc.is_finalized():
        nc.finalize()
    return nc


def kernel(inputs: np.ndarray, W: np.ndarray) -> np.ndarray:
    from concourse.bass_utils import run_bass_kernel_spmd

    nc = build_bass()
    x = np.ascontiguousarray(np.asarray(inputs, dtype=np.float32))
    w = np.ascontiguousarray(np.asarray(W, dtype=np.float32))
    in_maps = [{"x": x[i], "W": w} for i in range(B)]
    res = run_bass_kernel_spmd(nc, in_maps, core_ids=list(range(B)))
    out = np.stack([res.results[i]["out"] for i in range(B)], axis=0)
    return out.astype(np.float32)


if __name__ == "__main__":
    rng = np.random.default_rng(0)
    x = rng.standard_normal((B, T, D), dtype=np.float32)
    w = (rng.standard_normal((D, D)) * 0.05).astype(np.float32)
    out = kernel(inputs=x, W=w)
    print("out", out.shape, out.dtype)
